# revision 2
# baseline (speedup 1.0000x reference)
"""Multi-head causal attention on 8 TRN2 NeuronCores.

Sharding: core c -> batch c//2, head-group c%2 (8 of 16 heads).
Wq/Wk/Wv column-sharded, Wo row-sharded; the Wo all-reduce is the host-side
sum of the two partial outputs per batch.

Per-core kernel (Bass/Tile):
  phase A: kT = Wk^T x_k^T [512, 2048] (transposed), v = x_v Wv [2048, 8, 65]
           (65th column per head = ones -> softmax denominator via PV matmul)
  per sq-tile t (512 queries):
    qT_t = (Wq*s)^T x_q^T slice [512, 512]
    head pairs (PE row groups 0-63/64-127 run QK concurrently):
      attnT chunks [sk 128, sq 512] = kT^T qT, exp on ACT, causal zeroing via
      gpsimd affine_select (identity-matmul additive mask for general masks),
      PV accumulates out^T[65, 512] (row 64 = denominator).
    epilogue (deferred one pair): reciprocal -> PE broadcast -> normalize,
    DMA into aoT. Output projection of tile t-1 interleaved into tile t's
    attention to fill PE gaps.
"""

import os
import sys

for _p in ("/opt/trn_rl_repo", "/root/.axon_site/_ro/trn_rl_repo"):
    if os.path.isdir(_p) and _p not in sys.path:
        sys.path.insert(0, _p)

import numpy as np


def _ensure_axon_hooks():
    """Provide antenv.axon_hooks if the image lacks it (needed only when
    BASS_TRACE profiling is requested; degrades to no-trace on failure)."""
    try:
        import antenv.axon_hooks  # noqa: F401

        return
    except ImportError:
        pass
    try:
        import types

        import antenv

        mod = types.ModuleType("antenv.axon_hooks")
        holder = [None]

        def set_axon_ntff_profile_hook(h):
            holder[0] = h

        def get_axon_ntff_profile_hook():
            if holder[0] is None:
                try:
                    from trn_agent_boot.trn_boot import _ntff_profile_via_ctypes

                    holder[0] = _ntff_profile_via_ctypes("/opt/axon/libaxon_pjrt.so")
                except Exception:
                    return None
            return holder[0]

        mod.set_axon_ntff_profile_hook = set_axon_ntff_profile_hook
        mod.get_axon_ntff_profile_hook = get_axon_ntff_profile_hook
        sys.modules["antenv.axon_hooks"] = mod
        antenv.axon_hooks = mod
    except Exception:
        pass


_ensure_axon_hooks()

import concourse.bass as bass  # noqa: F401
import concourse.tile as tile
from concourse import bacc, mybir
from concourse.bass_utils import run_bass_kernel_spmd

F32 = mybir.dt.float32
F32R = mybir.dt.float32r
BF16 = mybir.dt.bfloat16

B, S, D = 4, 2048, 1024
H, DH = 16, 64
SCALE = DH**-0.5
NCORES = 8
NHPC = 8
HDPC = NHPC * DH  # 512
SQT = 512
NSQT = S // SQT  # 4
SKC = 128
NSKC = S // SKC  # 16
NDC = D // 128  # 8
NMC = HDPC // 128  # 4
NEG = -1.0e30

CFG = {
    "qk": "f32r",
    "pv": "f32r",
    "ao": "f32r",
    "xbufs": 2,
    "qtbufs": 2,
    "ebufs": 5,
    "sbufs": 2,
    "psq": 3,
    "psv": 4,
}

LAST_RESULTS = None

_DT = {"f32r": F32R, "bf16": BF16, "fp16": mybir.dt.float16}


def _mask_layout(mask: np.ndarray):
    """Blocks of [sk=128, sq=512].  Returns chunks[t] = list of
    (c, kind, arg): kind 'clear' (no masking), 'affine' (causal-style
    triangle, arg = affine base), or 'madd' (arg = packed additive tile idx).
    Fully-masked blocks are dropped."""
    chunks = []
    uniq = {}
    madds = []
    rr = np.arange(SKC)[:, None]
    jj = np.arange(SQT)[None, :]
    for t in range(NSQT):
        lst = []
        for c in range(NSKC):
            sub = mask[t * SQT : (t + 1) * SQT, c * SKC : (c + 1) * SKC]
            if sub.all():
                continue
            if not sub.any():
                lst.append((c, "clear", 0))
                continue
            subT = sub.T
            base = c * SKC - t * SQT
            if np.array_equal(subT, (rr + base) > jj):
                lst.append((c, "affine", base))
                continue
            key = subT.tobytes()
            if key not in uniq:
                madds.append(np.where(subT, NEG, 0.0).astype(np.float32))
                uniq[key] = len(madds) - 1
            lst.append((c, "madd", uniq[key]))
        assert lst, f"sq tile {t} fully masked"
        chunks.append(lst)
    madd_arr = (
        np.stack(madds) if madds else np.zeros((1, SKC, SQT), dtype=np.float32)
    )
    return chunks, madd_arr, bool(madds)


def _build_program(chunks, n_madd, use_madd, cfg, tick=False, reps=1):
    qk_dt = _DT[cfg["qk"]]
    pv_dt = _DT[cfg["pv"]]
    ao_dt = _DT[cfg["ao"]]

    nc = bacc.Bacc(
        "TRN2", target_bir_lowering=False, debug=False, num_devices=NCORES
    )
    if tick:
        tick_ap = nc.dram_tensor("tick", [128, 8], F32, kind="ExternalInput").ap()
    xqT = nc.dram_tensor("xqT", [D, S], F32, kind="ExternalInput").ap()
    xkT = nc.dram_tensor("xkT", [D, S], F32, kind="ExternalInput").ap()
    xvT = nc.dram_tensor("xvT", [D, S], F32, kind="ExternalInput").ap()
    wq = nc.dram_tensor("wq", [D, HDPC], F32, kind="ExternalInput").ap()
    wk = nc.dram_tensor("wk", [D, HDPC], F32, kind="ExternalInput").ap()
    wv = nc.dram_tensor("wv", [D, HDPC], F32, kind="ExternalInput").ap()
    wo = nc.dram_tensor("wo", [HDPC, D], F32, kind="ExternalInput").ap()
    if use_madd:
        madd = nc.dram_tensor(
            "madd", [n_madd, SKC, SQT], F32, kind="ExternalInput"
        ).ap()
    out = nc.dram_tensor("out", [S, D], F32, kind="ExternalOutput").ap()

    with tile.TileContext(nc) as tc:
        with (
            tc.tile_pool(name="const", bufs=1) as const,
            tc.tile_pool(name="wbig", bufs=2) as wbig,
            tc.tile_pool(name="xpool", bufs=cfg["xbufs"]) as xpool,
            tc.tile_pool(name="qtp", bufs=cfg["qtbufs"]) as qtp,
            tc.tile_pool(name="aop", bufs=2) as aop,
            tc.tile_pool(name="big", bufs=1) as big,
            tc.tile_pool(name="epool", bufs=cfg["ebufs"]) as epool,
            tc.tile_pool(name="spool", bufs=cfg["sbufs"]) as spool,
            tc.tile_pool(name="opool", bufs=2) as opool,
            tc.tile_pool(name="psq", bufs=cfg["psq"], space="PSUM") as psq,
            tc.tile_pool(name="psv", bufs=cfg["psv"], space="PSUM") as psv,
            tc.tile_pool(name="pso", bufs=1, space="PSUM") as pso,
        ):
            # ---- constants ----
            if tick:
                tick_sb = const.tile([128, 8], F32)
                nc.sync.dma_start(tick_sb, tick_ap)
            ones_plane = const.tile([128, 128], F32)
            nc.vector.memset(ones_plane, 1.0)
            ones65 = const.tile([65, 64], F32R)
            o65f = const.tile([65, 64], F32)
            nc.vector.memset(o65f, 1.0)
            nc.vector.tensor_copy(ones65, o65f)
            if use_madd:
                ident_sb = const.tile([128, 128], BF16)
                nc.gpsimd.memset(ident_sb, 0.0)
                nc.gpsimd.affine_select(
                    out=ident_sb,
                    in_=ident_sb,
                    compare_op=mybir.AluOpType.not_equal,
                    fill=1.0,
                    base=0,
                    pattern=[[-1, 128]],
                    channel_multiplier=1,
                )
                madd_sb = const.tile([SKC, n_madd, SQT], BF16)
                nc.gpsimd.dma_start(madd_sb, madd.rearrange("n p s -> p n s"))
            wo_sb = const.tile([128, NMC, D], ao_dt)
            if ao_dt == F32R:
                nc.sync.dma_start(
                    wo_sb, wo.rearrange("(c p) m -> p c m", p=128).bitcast(F32R)
                )
            else:
                nc.gpsimd.dma_start(wo_sb, wo.rearrange("(c p) m -> p c m", p=128))

            def emit_body():
                # ---- persistent tiles ----
                kT_sb = big.tile([128, NMC, S], qk_dt, tag="kT")
                v_sb = big.tile([128, NSKC, NHPC, DH + 1], pv_dt, tag="v")
                nc.vector.tensor_copy(
                    v_sb[:, :, :, DH : DH + 1],
                    ones_plane.rearrange("p (a b c) -> p a b c", a=NSKC, b=NHPC),
                )

                def load_xT(src, n):
                    xt = xpool.tile([128, NDC, SQT], F32R, tag="xt")
                    nc.sync.dma_start(
                        xt,
                        src[:, n * SQT : (n + 1) * SQT]
                        .rearrange("(c p) s -> p c s", p=128)
                        .bitcast(F32R),
                    )
                    return xt

                def load_w(wsrc):
                    wt = wbig.tile([128, NDC, HDPC], F32R, tag="wt")
                    nc.sync.dma_start(
                        wt, wsrc.rearrange("(c p) m -> p c m", p=128).bitcast(F32R)
                    )
                    return wt

                # ---- phase A: kT and v ----
                wt = load_w(wk)
                for n in range(NSQT):
                    xt = load_xT(xkT, n)
                    for m in range(NMC):
                        ps = psq.tile([128, SQT], F32, tag="ps512")
                        for kc in range(NDC):
                            nc.tensor.matmul(
                                ps,
                                wt[:, kc, m * 128 : (m + 1) * 128],
                                xt[:, kc, :],
                                start=(kc == 0),
                                stop=(kc == NDC - 1),
                            )
                        nc.vector.tensor_copy(kT_sb[:, m, n * SQT : (n + 1) * SQT], ps)

                wt = load_w(wv)
                for n in range(NSQT):
                    xt = load_xT(xvT, n)
                    for si in range(4):
                        sc = n * 4 + si
                        ps = psq.tile([128, SQT], F32, tag="ps512")
                        for kc in range(NDC):
                            nc.tensor.matmul(
                                ps,
                                xt[:, kc, si * 128 : (si + 1) * 128],
                                wt[:, kc, :],
                                start=(kc == 0),
                                stop=(kc == NDC - 1),
                            )
                        nc.vector.tensor_copy(
                            v_sb[:, sc, :, 0:DH],
                            ps.rearrange("p (h e) -> p h e", h=NHPC),
                        )

                # ---- deferred work helpers ----
                def epilogue(pv, h, aoT_t):
                    mc_ = h // 2
                    ro = (h % 2) * 64
                    den = spool.tile([65, SQT], F32R, tag="den")
                    with nc.allow_low_precision("softmax denom reciprocal"):
                        nc.vector.reciprocal(den[64:65, :], pv[64:65, :])
                    bc = psq.tile([64, SQT], F32, tag="ps512")
                    nc.tensor.matmul(bc, ones65[64:65, :], den[64:65, :])
                    bcast = spool.tile([64, SQT], F32, tag="bcast")
                    nc.scalar.copy(bcast, bc)
                    tmp = spool.tile([64, SQT], ao_dt, tag="tmp")
                    nc.vector.tensor_mul(tmp, pv[0:64, :], bcast)
                    nc.sync.dma_start(aoT_t[ro : ro + 64, mc_, :], tmp)

                def outproj_sc(aoT_prev, sc):
                    si = sc % 4
                    for j in range(2):
                        po = pso.tile([128, 512], F32, tag="po")
                        for mc2 in range(NMC):
                            nc.tensor.matmul(
                                po,
                                aoT_prev[:, mc2, si * 128 : (si + 1) * 128],
                                wo_sb[:, mc2, j * 512 : (j + 1) * 512],
                                start=(mc2 == 0),
                                stop=(mc2 == NMC - 1),
                            )
                        o_sb = opool.tile([128, 512], F32, tag="o")
                        nc.vector.tensor_copy(o_sb, po)
                        nc.sync.dma_start(
                            out[sc * 128 : (sc + 1) * 128, j * 512 : (j + 1) * 512],
                            o_sb,
                        )

                # ---- per sq-tile: qT_t, attention (head pairs), outproj(t-1) ----
                prev_epi = None  # (pvA, pvB, hA, hB, aoT_t)
                aoT_prev = None
                for t in range(NSQT):
                    xt = load_xT(xqT, t)
                    wtq = load_w(wq)
                    qT_t = qtp.tile([128, NMC, SQT], qk_dt, tag="qT")
                    for m in range(NMC):
                        ps = psq.tile([128, SQT], F32, tag="ps512")
                        for kc in range(NDC):
                            nc.tensor.matmul(
                                ps,
                                wtq[:, kc, m * 128 : (m + 1) * 128],
                                xt[:, kc, :],
                                start=(kc == 0),
                                stop=(kc == NDC - 1),
                            )
                        nc.vector.tensor_copy(qT_t[:, m, :], ps)
                        if m == 0 and prev_epi is not None:
                            pvA, pvB, hA, hB, ao_ = prev_epi
                            epilogue(pvA, hA, ao_)
                            epilogue(pvB, hB, ao_)
                            prev_epi = None

                    aoT_t = aop.tile([128, NMC, SQT], ao_dt, tag="aoT")
                    for hp in range(NHPC // 2):
                        hA, hB = 2 * hp, 2 * hp + 1
                        qsA = qT_t[0:64, hp, :]
                        qsB = qT_t[64:128, hp, :]
                        pvA = psv.tile([65, SQT], F32, tag="pv")
                        pvB = psv.tile([65, SQT], F32, tag="pv")
                        pend = None
                        for ci, (c, kind, arg) in enumerate(chunks[t]):
                            qkA = psq.tile([128, SQT], F32, tag="ps512")
                            qkB = psq.tile([128, SQT], F32, tag="ps512")
                            last = kind != "madd"
                            nc.tensor.matmul(
                                qkA,
                                kT_sb[0:64, hp, c * SKC : (c + 1) * SKC],
                                qsA,
                                start=True,
                                stop=last,
                            )
                            nc.tensor.matmul(
                                qkB,
                                kT_sb[64:128, hp, c * SKC : (c + 1) * SKC],
                                qsB,
                                start=True,
                                stop=last,
                            )
                            if kind == "madd":
                                nc.tensor.matmul(
                                    qkA, ident_sb, madd_sb[:, arg, :],
                                    start=False, stop=True,
                                )
                                nc.tensor.matmul(
                                    qkB, ident_sb, madd_sb[:, arg, :],
                                    start=False, stop=True,
                                )
                            eA = epool.tile([SKC, SQT], pv_dt, tag="e")
                            eB = epool.tile([SKC, SQT], pv_dt, tag="e")
                            nc.scalar.activation(
                                eA, qkA, mybir.ActivationFunctionType.Exp
                            )
                            nc.scalar.activation(
                                eB, qkB, mybir.ActivationFunctionType.Exp
                            )
                            if kind == "affine":
                                # keep e[r, j] iff j - r - base >= 0 (i.e. sk <= sq)
                                for e_ in (eA, eB):
                                    nc.gpsimd.affine_select(
                                        out=e_,
                                        in_=e_,
                                        compare_op=mybir.AluOpType.is_ge,
                                        fill=0.0,
                                        base=-arg,
                                        pattern=[[1, SQT]],
                                        channel_multiplier=-1,
                                    )
                            if pend is not None:
                                pc, peA, peB, pci = pend
                                nc.tensor.matmul(
                                    pvA, v_sb[:, pc, hA, :], peA,
                                    start=(pci == 0), stop=False,
                                )
                                nc.tensor.matmul(
                                    pvB, v_sb[:, pc, hB, :], peB,
                                    start=(pci == 0), stop=False,
                                )
                            pend = (c, eA, eB, ci)
                            if ci == 1 and prev_epi is not None:
                                ppA, ppB, phA, phB, ao_ = prev_epi
                                epilogue(ppA, phA, ao_)
                                epilogue(ppB, phB, ao_)
                                prev_epi = None
                        pc, peA, peB, pci = pend
                        nc.tensor.matmul(
                            pvA, v_sb[:, pc, hA, :], peA, start=(pci == 0), stop=True
                        )
                        nc.tensor.matmul(
                            pvB, v_sb[:, pc, hB, :], peB, start=(pci == 0), stop=True
                        )
                        prev_epi = (pvA, pvB, hA, hB, aoT_t)
                        if aoT_prev is not None:
                            outproj_sc(aoT_prev, (t - 1) * 4 + hp)
                    aoT_prev = aoT_t

                # flush: last pair epilogue + last tile's output projection
                pvA, pvB, hA, hB, ao_ = prev_epi
                epilogue(pvA, hA, ao_)
                epilogue(pvB, hB, ao_)
                for si in range(4):
                    outproj_sc(aoT_prev, (NSQT - 1) * 4 + si)

            for _rep in range(reps):
                emit_body()

    nc.finalize()
    return nc


_PROG_CACHE = {}


def kernel(x_q, x_k, x_v, mask, Wq, Wk, Wv, Wo):
    global LAST_RESULTS
    x_q = np.asarray(x_q, dtype=np.float32)
    x_k = np.asarray(x_k, dtype=np.float32)
    x_v = np.asarray(x_v, dtype=np.float32)
    mask = np.asarray(mask).astype(bool)
    Wq = np.asarray(Wq, dtype=np.float32)
    Wk = np.asarray(Wk, dtype=np.float32)
    Wv = np.asarray(Wv, dtype=np.float32)
    Wo = np.asarray(Wo, dtype=np.float32)

    chunks, madd_arr, use_madd = _mask_layout(mask)
    key = (
        tuple(tuple(lst) for lst in chunks),
        madd_arr.shape[0],
        use_madd,
        tuple(sorted(CFG.items())),
    )
    if key not in _PROG_CACHE:
        _PROG_CACHE[key] = _build_program(
            chunks, madd_arr.shape[0], use_madd, CFG
        )
    nc = _PROG_CACHE[key]

    wq_s = np.ascontiguousarray(Wq * np.float32(SCALE))
    in_maps = []
    for c in range(NCORES):
        b = c // 2
        hs = slice((c % 2) * HDPC, (c % 2 + 1) * HDPC)
        m = {
            "xqT": np.ascontiguousarray(x_q[b].T),
            "xkT": np.ascontiguousarray(x_k[b].T),
            "xvT": np.ascontiguousarray(x_v[b].T),
            "wq": np.ascontiguousarray(wq_s[:, hs]),
            "wk": np.ascontiguousarray(Wk[:, hs]),
            "wv": np.ascontiguousarray(Wv[:, hs]),
            "wo": np.ascontiguousarray(Wo[hs, :]),
        }
        if use_madd:
            m["madd"] = madd_arr
        in_maps.append(m)

    res = run_bass_kernel_spmd(nc, in_maps, core_ids=list(range(NCORES)))
    LAST_RESULTS = res
    out = np.empty((B, S, D), dtype=np.float32)
    for b in range(B):
        out[b] = res.results[2 * b]["out"] + res.results[2 * b + 1]["out"]
    return out



# revision 16
# speedup vs baseline: 1.8208x; 1.8208x over previous
"""Multi-head causal attention on 8 TRN2 NeuronCores.

Sharding: core c -> batch c//2, head-group c%2 (8 of 16 heads).
Wq/Wk/Wv column-sharded, Wo row-sharded; the Wo all-reduce is the host-side
sum of the two partial outputs per batch.

v2 design (per-core, Bass/Tile):
  - all matmul operands bf16 (v1 f32r lowered to fp32_mode=HIGH: 2-pass MMs,
    no FWL -> ~2.5x slower PE).  PSUM stays fp32.
  - attention blocks [sk 128, sq 512], attnT orientation; A/B head pair:
    QK row-group-concurrent MMs into one fused psum tile [128, 2, 512];
    ONE exp ACTIVATE per chunk over both halves (saves the 352-cycle
    per-instruction ACT overhead); causal chunks sliced to live columns
    [b, 512) on QK / exp / affine_select / PV.
  - softmax denominator via 65th ones-column of v (free on the PV stream).
  - epilogue: reciprocal_approx_fast (DVE, ~5x vs reciprocal) +
    gpsimd partition_broadcast + tensor_mul written straight into aoT.
  - single PSUM pool tag [128, 2, 512] x3 bufs (6 banks) shared by qk
    chunks, projection pairs and outproj pairs; pv pool [65, 512] x2.
  - projection / outproj work emitted as filler units interleaved between
    attention chunks so the PE never drains while ACT runs exp.
"""

import os
import sys

for _p in ("/opt/trn_rl_repo", "/root/.axon_site/_ro/trn_rl_repo"):
    if os.path.isdir(_p) and _p not in sys.path:
        sys.path.insert(0, _p)

import numpy as np


def _ensure_axon_hooks():
    """Provide antenv.axon_hooks if the image lacks it (needed only when
    BASS_TRACE profiling is requested; degrades to no-trace on failure)."""
    try:
        import antenv.axon_hooks  # noqa: F401

        return
    except ImportError:
        pass
    try:
        import types

        import antenv

        mod = types.ModuleType("antenv.axon_hooks")
        holder = [None]

        def set_axon_ntff_profile_hook(h):
            holder[0] = h

        def get_axon_ntff_profile_hook():
            if holder[0] is None:
                try:
                    from trn_agent_boot.trn_boot import _ntff_profile_via_ctypes

                    holder[0] = _ntff_profile_via_ctypes("/opt/axon/libaxon_pjrt.so")
                except Exception:
                    return None
            return holder[0]

        mod.set_axon_ntff_profile_hook = set_axon_ntff_profile_hook
        mod.get_axon_ntff_profile_hook = get_axon_ntff_profile_hook
        sys.modules["antenv.axon_hooks"] = mod
        antenv.axon_hooks = mod
    except Exception:
        pass


_ensure_axon_hooks()

import ml_dtypes

import concourse.bass as bass  # noqa: F401
import concourse.tile as tile
from concourse import bacc, mybir
from concourse.bass_utils import run_bass_kernel_spmd

F32 = mybir.dt.float32
BF16 = mybir.dt.bfloat16

B, S, D = 4, 2048, 1024
H, DH = 16, 64
SCALE = DH**-0.5
NCORES = 8
NHPC = 8
HDPC = NHPC * DH  # 512
SQT = 512
NSQT = S // SQT  # 4
SKC = 128
NSKC = S // SKC  # 16
NDC = D // 128  # 8
NMC = HDPC // 128  # 4
NEG = -1.0e30

CFG = {
    "dbg": False,
    "ps_bufs": 3,
    "pv_bufs": 2,
    "ebufs": 4,
}

LAST_RESULTS = None


def _mask_layout(mask: np.ndarray):
    """Blocks of [sk=128, sq=512].  Returns chunks[t] = list of
    (c, kind, arg, b): kind 'clear' (no masking), 'affine' (causal-style
    triangle, arg = affine base, b = first live column), or 'madd'
    (arg = packed additive tile idx, b = 0).  Fully-masked blocks dropped."""
    chunks = []
    uniq = {}
    madds = []
    rr = np.arange(SKC)[:, None]
    jj = np.arange(SQT)[None, :]
    for t in range(NSQT):
        lst = []
        for c in range(NSKC):
            sub = mask[t * SQT : (t + 1) * SQT, c * SKC : (c + 1) * SKC]
            if sub.all():
                continue
            if not sub.any():
                lst.append((c, "clear", 0, 0))
                continue
            subT = sub.T
            base = c * SKC - t * SQT
            if np.array_equal(subT, (rr + base) > jj):
                # column j has a live key iff j >= base
                lst.append((c, "affine", base, max(0, base)))
                continue
            key = subT.tobytes()
            if key not in uniq:
                madds.append(np.where(subT, NEG, 0.0).astype(np.float32))
                uniq[key] = len(madds) - 1
            lst.append((c, "madd", uniq[key], 0))
        assert lst, f"sq tile {t} fully masked"
        assert lst[0][3] == 0, f"sq tile {t}: first chunk must be full-width"
        chunks.append(lst)
    madd_arr = (
        np.stack(madds) if madds else np.zeros((1, SKC, SQT), dtype=np.float32)
    )
    return chunks, madd_arr, bool(madds)


def _build_program(chunks, n_madd, use_madd, cfg):
    nc = bacc.Bacc(
        "TRN2", target_bir_lowering=False, debug=False, num_devices=NCORES
    )
    xqT = nc.dram_tensor("xqT", [D, S], BF16, kind="ExternalInput").ap()
    xkT = nc.dram_tensor("xkT", [D, S], BF16, kind="ExternalInput").ap()
    xvT = nc.dram_tensor("xvT", [D, S], BF16, kind="ExternalInput").ap()
    wq = nc.dram_tensor("wq", [D, HDPC], BF16, kind="ExternalInput").ap()
    wk = nc.dram_tensor("wk", [D, HDPC], BF16, kind="ExternalInput").ap()
    wv = nc.dram_tensor("wv", [D, HDPC], BF16, kind="ExternalInput").ap()
    wo = nc.dram_tensor("wo", [HDPC, D], BF16, kind="ExternalInput").ap()
    if use_madd:
        madd = nc.dram_tensor(
            "madd", [n_madd, SKC, SQT], BF16, kind="ExternalInput"
        ).ap()
    out = nc.dram_tensor("out", [S, D], F32, kind="ExternalOutput").ap()
    if cfg.get("dbg"):
        dbg_pv = nc.dram_tensor(
            "dbg_pv", [8, 65, SQT], F32, kind="ExternalOutput"
        ).ap()
        dbg_bc = nc.dram_tensor(
            "dbg_bc", [8, 64, SQT], F32, kind="ExternalOutput"
        ).ap()
        dbg_ao = nc.dram_tensor(
            "dbg_ao", [NSQT, 128, NMC, SQT], BF16, kind="ExternalOutput"
        ).ap()

    with tile.TileContext(nc) as tc:
        with (
            tc.tile_pool(name="const", bufs=1) as const,
            tc.tile_pool(name="big", bufs=1) as big,
            tc.tile_pool(name="xpool", bufs=3) as xpool,
            tc.tile_pool(name="qtp", bufs=2) as qtp,
            tc.tile_pool(name="aop", bufs=2) as aop,
            tc.tile_pool(name="epool", bufs=cfg["ebufs"]) as epool,
            tc.tile_pool(name="spool", bufs=3) as spool,
            tc.tile_pool(name="opool", bufs=2) as opool,
            tc.tile_pool(name="psp", bufs=cfg["ps_bufs"], space="PSUM") as psp,
            tc.tile_pool(name="psv", bufs=cfg["pv_bufs"], space="PSUM") as psv,
        ):
            # ---- persistent weights ----
            wq_sb = const.tile([128, NDC, HDPC], BF16, tag="wq")
            wk_sb = const.tile([128, NDC, HDPC], BF16, tag="wk")
            wv_sb = const.tile([128, NDC, HDPC], BF16, tag="wv")
            wo_sb = const.tile([128, NMC, D], BF16, tag="wo")
            for wtile, wsrc in ((wk_sb, wk), (wv_sb, wv), (wq_sb, wq)):
                nc.sync.dma_start(
                    wtile, wsrc.rearrange("(c p) m -> p c m", p=128)
                )
            nc.sync.dma_start(wo_sb, wo.rearrange("(c p) m -> p c m", p=128))
            if use_madd:
                ident_sb = const.tile([128, 128], BF16)
                nc.gpsimd.memset(ident_sb, 0.0)
                nc.gpsimd.affine_select(
                    out=ident_sb,
                    in_=ident_sb,
                    compare_op=mybir.AluOpType.not_equal,
                    fill=1.0,
                    base=0,
                    pattern=[[-1, 128]],
                    channel_multiplier=1,
                )
                madd_sb = const.tile([SKC, n_madd, SQT], BF16)
                nc.gpsimd.dma_start(madd_sb, madd.rearrange("n p s -> p n s"))

            # ---- persistent kT / v ----
            kT_sb = big.tile([128, NMC, S], BF16, tag="kT")
            v_sb = big.tile([128, NSKC, NHPC, DH + 1], BF16, tag="v")
            ones_plane = const.tile([128, 128], BF16)
            nc.vector.memset(ones_plane, 1.0)
            nc.vector.tensor_copy(
                v_sb[:, :, :, DH : DH + 1],
                ones_plane.rearrange("p (a b c) -> p a b c", a=NSKC, b=NHPC),
            )

            def load_xT(src, n):
                xt = xpool.tile([128, NDC, SQT], BF16, tag="xt")
                nc.sync.dma_start(
                    xt,
                    src[:, n * SQT : (n + 1) * SQT].rearrange(
                        "(c p) s -> p c s", p=128
                    ),
                )
                return xt

            # ---------- filler units (projection / outproj micro-ops) ----
            # each unit is a closure emitting ~4 matmuls (+ cast/DMA);
            # drained between attention chunks to keep PE fed while ACT
            # runs exp.

            def kq_proj_units(wt, xt_box, dst, dst_n, src, n):
                """kT- or qT-projection for sq-tile n: units for m-pairs.
                dst[:, m:m+2, dst_n*512...] <- (w.T x)."""
                units = []

                def load():
                    xt_box[0] = load_xT(src, n)

                units.append(load)
                for m0 in (0, 2):
                    box = {}

                    def h1(m0=m0, box=box):
                        ps = psp.tile([128, 2, SQT], F32, tag="ps")
                        box["ps"] = ps
                        xt = xt_box[0]
                        for half in range(2):
                            m = m0 + half
                            for kc in range(4):
                                nc.tensor.matmul(
                                    ps[:, half, :],
                                    wt[:, kc, m * 128 : (m + 1) * 128],
                                    xt[:, kc, :],
                                    start=(kc == 0),
                                    stop=False,
                                )

                    def h2(m0=m0, box=box):
                        ps = box["ps"]
                        xt = xt_box[0]
                        for half in range(2):
                            m = m0 + half
                            for kc in range(4, NDC):
                                nc.tensor.matmul(
                                    ps[:, half, :],
                                    wt[:, kc, m * 128 : (m + 1) * 128],
                                    xt[:, kc, :],
                                    start=False,
                                    stop=(kc == NDC - 1),
                                )
                        nc.vector.tensor_copy(
                            dst[:, m0 : m0 + 2, dst_n * SQT : (dst_n + 1) * SQT],
                            ps,
                        )

                    units.append(h1)
                    units.append(h2)
                return units

            def v_proj_units(xt_box, n):
                units = []

                def load():
                    xt_box[0] = load_xT(xvT, n)

                units.append(load)
                for si0 in (0, 2):
                    box = {}

                    def h1(si0=si0, box=box):
                        ps = psp.tile([128, 2, SQT], F32, tag="ps")
                        box["ps"] = ps
                        xt = xt_box[0]
                        for half in range(2):
                            si = si0 + half
                            for kc in range(4):
                                nc.tensor.matmul(
                                    ps[:, half, :],
                                    xt[:, kc, si * 128 : (si + 1) * 128],
                                    wv_sb[:, kc, :],
                                    start=(kc == 0),
                                    stop=False,
                                )

                    def h2(si0=si0, box=box, n=n):
                        ps = box["ps"]
                        xt = xt_box[0]
                        for half in range(2):
                            si = si0 + half
                            for kc in range(4, NDC):
                                nc.tensor.matmul(
                                    ps[:, half, :],
                                    xt[:, kc, si * 128 : (si + 1) * 128],
                                    wv_sb[:, kc, :],
                                    start=False,
                                    stop=(kc == NDC - 1),
                                )
                        nc.vector.tensor_copy(
                            v_sb[:, n * 4 + si0 : n * 4 + si0 + 2, :, 0:DH],
                            ps.rearrange("p a (h e) -> p a h e", h=NHPC),
                        )

                    units.append(h1)
                    units.append(h2)
                return units

            def outproj_units(aoT_prev, t_prev):
                units = []
                for si in range(4):
                    sc = t_prev * 4 + si
                    box = {}

                    def h1(si=si, box=box, aoT_prev=aoT_prev):
                        po = psp.tile([128, 2, SQT], F32, tag="ps")
                        box["po"] = po
                        for j in range(2):
                            for mc2 in range(2):
                                nc.tensor.matmul(
                                    po[:, j, :],
                                    aoT_prev[:, mc2, si * 128 : (si + 1) * 128],
                                    wo_sb[:, mc2, j * 512 : (j + 1) * 512],
                                    start=(mc2 == 0),
                                    stop=False,
                                )

                    def h2(si=si, sc=sc, box=box, aoT_prev=aoT_prev):
                        po = box["po"]
                        for j in range(2):
                            for mc2 in range(2, NMC):
                                nc.tensor.matmul(
                                    po[:, j, :],
                                    aoT_prev[:, mc2, si * 128 : (si + 1) * 128],
                                    wo_sb[:, mc2, j * 512 : (j + 1) * 512],
                                    start=False,
                                    stop=(mc2 == NMC - 1),
                                )
                        o_sb = opool.tile([128, 2, SQT], F32, tag="o")
                        nc.vector.tensor_copy(o_sb, po)
                        nc.sync.dma_start(
                            out[sc * 128 : (sc + 1) * 128, :],
                            o_sb.rearrange("p a m -> p (a m)"),
                        )

                    units.append(h1)
                    units.append(h2)
                return units

            # ---------- epilogue ----------
            # HW-probed constraints: custom/extended ops (recip_approx_fast,
            # partition_broadcast) require partition-base-0 APs; standard DVE
            # ops (tensor_copy/tensor_mul) may shift partition bases freely.
            def evac_pv(pv):
                """Evacuate a finished pv psum tile to SBUF (frees the bank):
                dh rows to pv_sb, denominator row shifted to partition 0."""
                pv_sb = spool.tile([64, SQT], F32, tag="pvs", bufs=4)
                nc.vector.tensor_copy(pv_sb, pv[0:64, :])
                den0 = spool.tile([1, SQT], F32, tag="den0", bufs=4)
                nc.vector.tensor_copy(den0, pv[64:65, :])
                return pv_sb, den0

            def epilogue(pvd, h, aoT_t, dbg=None):
                pv_sb, den0 = pvd
                mc_ = h // 2
                ro = (h % 2) * 64
                denr = spool.tile([1, SQT], F32, tag="denr")
                nc.vector.reciprocal_approx_fast(denr, den0)
                bcast = spool.tile([64, SQT], F32, tag="bcast")
                nc.gpsimd.partition_broadcast(bcast, denr)
                nc.vector.tensor_mul(aoT_t[ro : ro + 64, mc_, :], pv_sb, bcast)
                if dbg is not None:
                    i = dbg
                    nc.sync.dma_start(dbg_pv[i, 0:64], pv_sb)
                    nc.sync.dma_start(dbg_pv[i, 64:65], denr)
                    nc.sync.dma_start(dbg_bc[i], bcast)

            # ---------- prologue: kT/v/qT for sq-tile 0 (serial) ----------
            xk_box, xv_box, xq_box = [None], [None], [None]
            for u in kq_proj_units(wk_sb, xk_box, kT_sb, 0, xkT, 0):
                u()
            for u in v_proj_units(xv_box, 0):
                u()
            qT_tiles = [None] * NSQT
            qT_tiles[0] = qtp.tile([128, NMC, SQT], BF16, tag="qT", name="qT0")
            for u in kq_proj_units(
                wq_sb, xq_box, qT_tiles[0], 0, xqT, 0
            ):
                u()

            # ---------- main loop ----------
            aoT_prev = None
            for t in range(NSQT):
                # build filler queue for this tile
                filler = []
                if t + 1 < NSQT:
                    xkb, xvb, xqb = [None], [None], [None]
                    filler += kq_proj_units(
                        wk_sb, xkb, kT_sb, t + 1, xkT, t + 1
                    )
                    filler += v_proj_units(xvb, t + 1)
                    qT_tiles[t + 1] = qtp.tile([128, NMC, SQT], BF16, tag="qT", name=f"qT{t+1}")
                    filler += kq_proj_units(
                        wq_sb, xqb, qT_tiles[t + 1], 0, xqT, t + 1
                    )
                if aoT_prev is not None:
                    filler += outproj_units(aoT_prev, t - 1)
                filler.reverse()  # pop() from the front-emitted end

                n_slots = max(1, len(chunks[t]) * 4)
                per_slot = max(1, -(-len(filler) // n_slots))

                qT_t = qT_tiles[t]
                aoT_t = aop.tile([128, NMC, SQT], BF16, tag="aoT")
                for hp in range(NHPC // 2):
                    hA, hB = 2 * hp, 2 * hp + 1
                    pvA = psv.tile([65, SQT], F32, tag="pv")
                    pvB = psv.tile([65, SQT], F32, tag="pv")
                    pend = None
                    for ci, (c, kind, arg, b) in enumerate(chunks[t]):
                        qk = psp.tile([128, 2, SQT], F32, tag="ps")
                        last = kind != "madd"
                        nc.tensor.matmul(
                            qk[:, 0, b:],
                            kT_sb[0:64, hp, c * SKC : (c + 1) * SKC],
                            qT_t[0:64, hp, b:],
                            start=True,
                            stop=last,
                        )
                        nc.tensor.matmul(
                            qk[:, 1, b:],
                            kT_sb[64:128, hp, c * SKC : (c + 1) * SKC],
                            qT_t[64:128, hp, b:],
                            start=True,
                            stop=last,
                        )
                        if kind == "madd":
                            for half in range(2):
                                nc.tensor.matmul(
                                    qk[:, half, :],
                                    ident_sb,
                                    madd_sb[:, arg, :],
                                    start=False,
                                    stop=True,
                                )
                        e = epool.tile([SKC, 2, SQT], BF16, tag="e")
                        nc.scalar.activation(
                            e[:, :, b:],
                            qk[:, :, b:],
                            mybir.ActivationFunctionType.Exp,
                        )
                        if kind == "affine":
                            nc.gpsimd.affine_select(
                                out=e[:, :, b:],
                                in_=e[:, :, b:],
                                compare_op=mybir.AluOpType.is_ge,
                                fill=0.0,
                                base=b - arg,
                                pattern=[[0, 2], [1, SQT - b]],
                                channel_multiplier=-1,
                            )
                        for _ in range(per_slot):
                            if filler:
                                filler.pop()()
                        if pend is not None:
                            pc, pe_, pb, pci = pend
                            nc.tensor.matmul(
                                pvA[:, pb:],
                                v_sb[:, pc, hA, :],
                                pe_[:, 0, pb:],
                                start=(pci == 0),
                                stop=False,
                            )
                            nc.tensor.matmul(
                                pvB[:, pb:],
                                v_sb[:, pc, hB, :],
                                pe_[:, 1, pb:],
                                start=(pci == 0),
                                stop=False,
                            )
                        pend = (c, e, b, ci)
                    pc, pe_, pb, pci = pend
                    nc.tensor.matmul(
                        pvA[:, pb:],
                        v_sb[:, pc, hA, :],
                        pe_[:, 0, pb:],
                        start=(pci == 0),
                        stop=True,
                    )
                    nc.tensor.matmul(
                        pvB[:, pb:],
                        v_sb[:, pc, hB, :],
                        pe_[:, 1, pb:],
                        start=(pci == 0),
                        stop=True,
                    )
                    di = t * 8 + 2 * hp if cfg.get("dbg") and t == 0 else None
                    epilogue(
                        evac_pv(pvA), hA, aoT_t,
                        dbg=di if di is not None else None,
                    )
                    epilogue(
                        evac_pv(pvB), hB, aoT_t,
                        dbg=(di + 1) if di is not None else None,
                    )
                # drain any leftover filler before next tile
                while filler:
                    filler.pop()()
                if cfg.get("dbg"):
                    nc.sync.dma_start(dbg_ao[t], aoT_t)
                aoT_prev = aoT_t

            # flush: last tile's output projection
            for u in outproj_units(aoT_prev, NSQT - 1):
                u()

    nc.finalize()
    return nc


_PROG_CACHE = {}


def kernel(x_q, x_k, x_v, mask, Wq, Wk, Wv, Wo):
    global LAST_RESULTS
    x_q = np.asarray(x_q, dtype=np.float32)
    x_k = np.asarray(x_k, dtype=np.float32)
    x_v = np.asarray(x_v, dtype=np.float32)
    mask = np.asarray(mask).astype(bool)
    Wq = np.asarray(Wq, dtype=np.float32)
    Wk = np.asarray(Wk, dtype=np.float32)
    Wv = np.asarray(Wv, dtype=np.float32)
    Wo = np.asarray(Wo, dtype=np.float32)

    chunks, madd_arr, use_madd = _mask_layout(mask)
    key = (
        tuple(tuple(lst) for lst in chunks),
        madd_arr.shape[0],
        use_madd,
        tuple(sorted((k, str(v)) for k, v in CFG.items())),
    )
    if key not in _PROG_CACHE:
        _PROG_CACHE[key] = _build_program(
            chunks, madd_arr.shape[0], use_madd, CFG
        )
    nc = _PROG_CACHE[key]

    bf = ml_dtypes.bfloat16
    wq_s = (Wq * np.float32(SCALE)).astype(bf)
    wk_b = Wk.astype(bf)
    wv_b = Wv.astype(bf)
    wo_b = Wo.astype(bf)
    madd_b = madd_arr.astype(bf)
    in_maps = []
    for c in range(NCORES):
        b = c // 2
        hs = slice((c % 2) * HDPC, (c % 2 + 1) * HDPC)
        m = {
            "xqT": np.ascontiguousarray(x_q[b].T.astype(bf)),
            "xkT": np.ascontiguousarray(x_k[b].T.astype(bf)),
            "xvT": np.ascontiguousarray(x_v[b].T.astype(bf)),
            "wq": np.ascontiguousarray(wq_s[:, hs]),
            "wk": np.ascontiguousarray(wk_b[:, hs]),
            "wv": np.ascontiguousarray(wv_b[:, hs]),
            "wo": np.ascontiguousarray(wo_b[hs, :]),
        }
        if use_madd:
            m["madd"] = madd_b
        in_maps.append(m)

    res = run_bass_kernel_spmd(nc, in_maps, core_ids=list(range(NCORES)))
    LAST_RESULTS = res
    out = np.empty((B, S, D), dtype=np.float32)
    for b in range(B):
        out[b] = res.results[2 * b]["out"] + res.results[2 * b + 1]["out"]
    return out
